# revision 8
# baseline (speedup 1.0000x reference)
"""Distributed Trainium2 kernel for MQA causal attention (B=2, S=2048, D=2048,
N=8 query heads, K=1 KV head, H=256), sharded over 8 NeuronCores.

Sharding (SPMD-uniform, identical graph on every core):
  - Tensor-parallel over the 8 query heads: core n owns head n for BOTH batches.
  - KV projection data-parallel over the 4096 flattened tokens (512/core),
    followed by an 8-rank AllGather of the rope'd K (transposed) and V.
  - After attention, the enc tensor is re-sharded head-split -> token-split via
    TWO half-H AllToAlls (so the second one overlaps the first wave of the
    output projection); core n emits output rows for global tokens
    [512n, 512n+512).

Schedule highlights (v2):
  - rope tables load at high priority so rope-k never waits on the x flood.
  - The AllGather input is packed into one [128,2048] tile -> one store with
    4KB descriptors; the collective triggers ~40us earlier than v1.
  - xt tiles stream in 2 chunks each so the DMA-engine FIFO backlog stays
    short (the AG input store has to drain through the same engines).
  - Attention chunk loop is software-pipelined with 2 chunks of logits
    lookahead over the scalar-engine exp latency.
  - Output projection runs in two db-waves of 8 PSUM tiles, chunk-outer, so
    accumulation consumes AllToAll#1's chunks while AllToAll#2 is in flight.
"""

from contextlib import ExitStack

import numpy as np
import ml_dtypes

import concourse.bacc as bacc
import concourse.bass as bass
import concourse.mybir as mybir
import concourse.tile as tile
from concourse.bass_utils import run_bass_kernel_spmd

BF = mybir.dt.bfloat16
F32 = mybir.dt.float32

NCORES = 8
B, S, D, N, H = 2, 2048, 2048, 8, 256
BT = B * S            # 4096 flattened tokens
TSH = BT // NCORES    # 512 tokens per core (kv shard / output shard)
HH = H // 2           # 128, rope half
NQB = S // 512        # 4 query blocks of 512 per batch
AluOp = mybir.AluOpType


def _build():
    nc = bacc.Bacc(
        "TRN2",
        target_bir_lowering=False,
        debug=False,
        enable_asserts=True,
        num_devices=NCORES,
    )

    # host-pre-laid-out inputs: partition-major SBUF tile images
    xTb = nc.dram_tensor("xTb", [8, 128, 8192], BF, kind="ExternalInput")
    xkv2 = nc.dram_tensor("xkv2", [128, 8192], BF, kind="ExternalInput")
    qw2 = nc.dram_tensor("qw2", [128, 4096], BF, kind="ExternalInput")
    kvw2 = nc.dram_tensor("kvw2", [128, 8192], BF, kind="ExternalInput")
    outw2 = nc.dram_tensor("outw2", [4, 128, 8192], BF, kind="ExternalInput")
    cosq = nc.dram_tensor("cosq", [HH, S], F32, kind="ExternalInput")
    sinq = nc.dram_tensor("sinq", [HH, S], F32, kind="ExternalInput")
    cosk = nc.dram_tensor("cosk", [HH, TSH], F32, kind="ExternalInput")
    sink = nc.dram_tensor("sink", [HH, TSH], F32, kind="ExternalInput")
    mask4 = nc.dram_tensor("mask4", [128, 2048], BF, kind="ExternalInput")
    out = nc.dram_tensor("out", [TSH, D], F32, kind="ExternalOutput")

    groups = [list(range(NCORES))]

    with tile.TileContext(nc) as tc, ExitStack() as es:
        consts = es.enter_context(tc.tile_pool(name="consts", bufs=1))

        def single(shape, dtype, name):
            return consts.tile(shape, dtype, name=name, tag=name)

        qw_sb = single([128, 16 * 256], BF, "qw_sb")
        cosq_sb = single([HH, S], F32, "cosq_sb")
        sinq_sb = single([HH, S], F32, "sinq_sb")
        cosk_sb = single([HH, TSH], F32, "cosk_sb")
        sink_sb = single([HH, TSH], F32, "sink_sb")
        mask_sb = single([128, 4 * 512], BF, "mask_sb")
        ones_sq = single([128, 128], BF, "ones_sq")
        qT_all = single([128, 2 * BT], BF, "qT_all")
        kT_sb = [single([128, BT], BF, f"kT{j}_sb") for j in range(2)]
        v_sb = single([128, (BT // 128) * 256], BF, "v_sb")
        enc_sb = [single([128, BT], BF, f"enc{j}_sb") for j in range(2)]

        psum = es.enter_context(tc.tile_pool(name="psum", bufs=8, space="PSUM"))
        bigp = es.enter_context(tc.tile_pool(name="bigp", bufs=1))
        xtp = es.enter_context(tc.tile_pool(name="xtp", bufs=3))
        tmpp = es.enter_context(tc.tile_pool(name="tmpp", bufs=6))
        stagep = es.enter_context(tc.tile_pool(name="stagep", bufs=1))
        ptp = es.enter_context(tc.tile_pool(name="ptp", bufs=8))
        rbp = es.enter_context(tc.tile_pool(name="rbp", bufs=3))
        osp = es.enter_context(tc.tile_pool(name="osp", bufs=4))
        dram = es.enter_context(tc.tile_pool(name="dram", bufs=1, space="DRAM"))

        kvw_sb = bigp.tile([128, 2 * 16 * 256], BF, name="kvw_sb", tag="big")

        kv_in = dram.tile([128, 2048], BF, name="kv_in", tag="kv_in")
        kv_all = dram.tile([NCORES, 128, 2048], BF, name="kv_all",
                           tag="kv_all", addr_space="Shared")
        enc_in = [dram.tile([NCORES, 128, 512], BF, name=f"enc_in{j}",
                            tag=f"enc_in{j}") for j in range(2)]
        enc_out = [dram.tile([NCORES, 128, 512], BF, name=f"enc_out{j}",
                             tag=f"enc_out{j}") for j in range(2)]

        # rope tables first: rope-k is on the AllGather critical path and must
        # not queue behind the x flood on the DMA engines.
        with tc.high_priority():
            nc.scalar.dma_start(cosk_sb[:], cosk[:])
            nc.scalar.dma_start(sink_sb[:], sink[:])
        nc.vector.memset(ones_sq[:], 1.0)

        # ---- KV projection over this core's 512-token shard ----
        ktp = [psum.tile([128, 512], F32, name=f"ktp{j}", tag="bank")
               for j in range(2)]
        vp = [psum.tile([128, 512], F32, name=f"vp{i}", tag="bank")
              for i in range(2)]
        xkt = xtp.tile([128, 16 * 512], BF, name="xkt", tag="xt")
        for c in range(4):
            k_sl = slice(c * 1024, (c + 1) * 1024)
            v_sl = slice(4096 + c * 1024, 4096 + (c + 1) * 1024)
            x_sl = slice(c * 2048, (c + 1) * 2048)
            nc.sync.dma_start(kvw_sb[:, k_sl], kvw2[:, k_sl])
            nc.sync.dma_start(kvw_sb[:, v_sl], kvw2[:, v_sl])
            nc.scalar.dma_start(xkt[:, x_sl], xkv2[:, x_sl])
        nc.gpsimd.dma_start(qw_sb[:], qw2[:])
        nc.scalar.dma_start(cosq_sb[:], cosq[:])
        nc.scalar.dma_start(sinq_sb[:], sinq[:])
        nc.scalar.dma_start(mask_sb[:], mask4[:])
        for dc in range(16):
            st, sp = dc == 0, dc == 15
            xk = xkt[:, dc * 512:(dc + 1) * 512]
            for j in range(2):
                nc.tensor.matmul(
                    ktp[j][:],
                    lhsT=kvw_sb[:, dc * 256 + j * 128:dc * 256 + (j + 1) * 128],
                    rhs=xk,
                    start=st, stop=sp,
                )
            for i in range(4):
                nc.tensor.matmul(
                    vp[i // 2][:, (i % 2) * 256:(i % 2 + 1) * 256],
                    lhsT=xkt[:, dc * 512 + i * 128:dc * 512 + (i + 1) * 128],
                    rhs=kvw_sb[:, 4096 + dc * 256:4096 + (dc + 1) * 256],
                    start=(st and i % 2 == 0),
                    stop=(sp and i % 2 == 1),
                )

        # rope k + stage v into one packed tile [k0|k1|v0|v1], single store
        # (4KB descriptor lines drain the congested engines 4x faster).
        kvpack = stagep.tile([128, 2048], BF, name="kvpack", tag="stage")
        for i in range(2):
            nc.vector.tensor_copy(kvpack[:, 1024 + i * 512:1024 + (i + 1) * 512],
                                  vp[i][:])
        t_a = tmpp.tile([128, 512], F32, name="t_a", tag="tmp")
        t_b = tmpp.tile([128, 512], F32, name="t_b", tag="tmp")
        nc.vector.tensor_mul(t_a[:], ktp[0][:], cosk_sb[:])
        nc.vector.tensor_mul(t_b[:], ktp[1][:], sink_sb[:])
        nc.vector.tensor_sub(kvpack[:, 0:512], t_a[:], t_b[:])
        t_c = tmpp.tile([128, 512], F32, name="t_c", tag="tmp")
        t_d = tmpp.tile([128, 512], F32, name="t_d", tag="tmp")
        nc.vector.tensor_mul(t_c[:], ktp[1][:], cosk_sb[:])
        nc.vector.tensor_mul(t_d[:], ktp[0][:], sink_sb[:])
        nc.vector.tensor_add(kvpack[:, 512:1024], t_c[:], t_d[:])
        nc.gpsimd.dma_start(kv_in[:], kvpack[:])

        nc.gpsimd.collective_compute(
            "AllGather",
            AluOp.bypass,
            replica_groups=groups,
            ins=[kv_in[:].opt()],
            outs=[kv_all[:].opt()],
        )

        # ---- gathered KV -> SBUF, batched per batch (3 dma_starts each) ----
        def load_kv_batch(b):
            sl = slice(b * 4, (b + 1) * 4)
            nc.scalar.dma_start(
                kT_sb[0][:, b * 2048:(b + 1) * 2048]
                .rearrange("p (s t) -> p s t", s=4),
                kv_all[sl, :, 0:512].rearrange("s p t -> p s t"),
            )
            nc.gpsimd.dma_start(
                kT_sb[1][:, b * 2048:(b + 1) * 2048]
                .rearrange("p (s t) -> p s t", s=4),
                kv_all[sl, :, 512:1024].rearrange("s p t -> p s t"),
            )
            nc.scalar.dma_start(
                v_sb[:, b * 4096:(b + 1) * 4096]
                .rearrange("p (s t) -> p s t", s=4),
                kv_all[sl, :, 1024:2048].rearrange("s p t -> p s t"),
            )

        def qproj_batch(b):
            """Project + rope this core's head over batch b's 2048 tokens."""
            for tb in range(b * 4, b * 4 + 4):
                qtp = [psum.tile([128, 512], F32, name=f"qtp{j}", tag="bank")
                       for j in range(2)]
                xt = xtp.tile([128, 16 * 512], BF, name="xt", tag="xt")
                # 2 chunks: finer DMA/compute interleave, 8KB lines
                for hc in range(2):
                    nc.sync.dma_start(xt[:, hc * 4096:(hc + 1) * 4096],
                                      xTb[tb, :, hc * 4096:(hc + 1) * 4096])
                for dc in range(16):
                    for j in range(2):
                        nc.tensor.matmul(
                            qtp[j][:],
                            lhsT=qw_sb[:, dc * 256 + j * 128:
                                       dc * 256 + (j + 1) * 128],
                            rhs=xt[:, dc * 512:(dc + 1) * 512],
                            start=dc == 0, stop=dc == 15,
                        )
                cq = cosq_sb[:, (tb % 4) * 512:(tb % 4 + 1) * 512]
                sq = sinq_sb[:, (tb % 4) * 512:(tb % 4 + 1) * 512]
                u_a = tmpp.tile([128, 512], F32, name="u_a", tag="tmp")
                u_b = tmpp.tile([128, 512], F32, name="u_b", tag="tmp")
                nc.vector.tensor_mul(u_a[:], qtp[0][:], cq)
                nc.vector.tensor_mul(u_b[:], qtp[1][:], sq)
                nc.vector.tensor_sub(
                    qT_all[:, tb * 512:(tb + 1) * 512], u_a[:], u_b[:]
                )
                u_c = tmpp.tile([128, 512], F32, name="u_c", tag="tmp")
                u_d = tmpp.tile([128, 512], F32, name="u_d", tag="tmp")
                nc.vector.tensor_mul(u_c[:], qtp[1][:], cq)
                nc.vector.tensor_mul(u_d[:], qtp[0][:], sq)
                nc.vector.tensor_add(
                    qT_all[:, BT + tb * 512:BT + (tb + 1) * 512],
                    u_c[:], u_d[:]
                )

        # ---- attention (causal): one 512-query block, SW-pipelined ----
        # Chunk ch attends 512 queries to keys [128ch, 128ch+128); the last 4
        # chunks are diagonal (matmuls shrink to causal width, in-chunk
        # triangle masked multiplicatively after exp). Logits run 2 chunks
        # ahead of sums/encp so the scalar-engine exp latency is hidden.
        def attn_block(b, qb):
            nch = 4 * (qb + 1)
            q0 = b * 2048 + qb * 512
            sums = psum.tile([128, 512], F32, name="sums", tag="bank")
            encp = [psum.tile([128, 512], F32, name=f"encp{j}", tag="bank")
                    for j in range(2)]

            def c0_of(ch):
                r = ch - (nch - 4)
                return max(r, 0) * 128

            stts = {}
            pts = {}

            def logits(ch):
                c0 = c0_of(ch)
                stt = psum.tile([128, 512], F32, name="stt", tag="bank")
                stts[ch] = stt
                k0 = b * 2048 + ch * 128
                for j in range(2):
                    nc.tensor.matmul(
                        stt[:, c0:],
                        lhsT=kT_sb[j][:, k0:k0 + 128],
                        rhs=qT_all[:, j * BT + q0 + c0:
                                   j * BT + q0 + 512],
                        start=j == 0, stop=j == 1,
                    )

            def exp_mask(ch):
                c0 = c0_of(ch)
                r = ch - (nch - 4)
                pt = ptp.tile([128, 512], BF, name="pt", tag="pt")
                pts[ch] = pt
                nc.scalar.activation(
                    pt[:, c0:], stts[ch][:, c0:],
                    mybir.ActivationFunctionType.Exp,
                )
                if r >= 0:
                    nc.vector.tensor_mul(
                        pt[:, c0:], pt[:, c0:],
                        mask_sb[:, r * 512 + c0:(r + 1) * 512],
                    )

            def accum(ch):
                c0 = c0_of(ch)
                pt = pts.pop(ch)
                stts.pop(ch)
                first, last = ch == 0, ch == nch - 1
                nc.tensor.matmul(
                    sums[:, c0:], lhsT=ones_sq[:], rhs=pt[:, c0:],
                    start=first, stop=last,
                )
                m = b * 16 + ch
                for j in range(2):
                    nc.tensor.matmul(
                        encp[j][:, c0:],
                        lhsT=v_sb[:, m * 256 + j * 128:
                                  m * 256 + (j + 1) * 128],
                        rhs=pt[:, c0:],
                        start=first, stop=last,
                    )

            logits(0)
            exp_mask(0)
            if nch > 1:
                logits(1)
                exp_mask(1)
            for ch in range(nch):
                if ch + 2 < nch:
                    logits(ch + 2)
                    exp_mask(ch + 2)
                accum(ch)

            rb_sb = rbp.tile([128, 512], F32, name="rb_sb", tag="rbs")
            nc.vector.reciprocal(rb_sb[:], sums[:])
            for j in range(2):
                nc.vector.tensor_mul(
                    enc_sb[j][:, q0:q0 + 512], encp[j][:], rb_sb[:],
                )
            for j in range(2):
                eng = nc.scalar if j == 0 else nc.gpsimd
                eng.dma_start(
                    enc_in[j][b * 4 + qb], enc_sb[j][:, q0:q0 + 512],
                )

        # ---- schedule ----
        qproj_batch(0)
        qproj_batch(1)
        load_kv_batch(0)
        load_kv_batch(1)
        for qb in range(NQB):
            attn_block(0, qb)
        for qb in range(NQB):
            attn_block(1, qb)

        # ---- two half-H AllToAlls: head-split -> token-split ----
        # out-proj weights stream on the idle sync queue during attention;
        # the 4th block waits for a pool slot and loads during wave B's db=2.
        oww = [None] * 4
        for db in range(3):
            oww[db] = xtp.tile([128, 16 * 512], BF, name=f"oww{db}", tag="xt")
            nc.sync.dma_start(oww[db][:], outw2[db])
        for j in range(2):
            nc.gpsimd.collective_compute(
                "AllToAll",
                AluOp.bypass,
                replica_groups=groups,
                ins=[enc_in[j][:].opt()],
                outs=[enc_out[j][:].opt()],
            )

        encf_sb = bigp.tile([128, 2 * 8 * 512], BF, name="encf_sb", tag="big")
        for j in range(2):
            eng = nc.scalar if j == 0 else nc.gpsimd
            eng.dma_start(
                encf_sb[:, j * 4096:(j + 1) * 4096]
                .rearrange("p (r t) -> p r t", r=8),
                enc_out[j][:].rearrange("r p t -> p r t"),
            )

        # ---- output projection: 2 waves of 8 PSUM tiles, chunk-outer ----
        # Wave accumulation order is (j=0 chunks, then j=1), so wave A starts
        # right after AllToAll#0 lands and chews j=0 chunks while AllToAll#1
        # is still in flight. lhsT is reused across consecutive db matmuls.
        def oproj_tile(op, db, tt, interleave_db=None):
            for j in range(2):
                for c in range(8):
                    col = j * 4096 + c * 512
                    lh = encf_sb[:, col + tt * 128:col + (tt + 1) * 128]
                    nc.tensor.matmul(
                        op[:], lhsT=lh,
                        rhs=oww[db][:, (j * 8 + c) * 512:(j * 8 + c + 1) * 512],
                        start=(j == 0 and c == 0), stop=(j == 1 and c == 7),
                    )

        def flush_tile(op, db, tt):
            o_sb = osp.tile([128, 512], F32, name="o_sb", tag="osb")
            nc.vector.tensor_copy(o_sb[:], op[:])
            eng = nc.scalar if (db + tt) % 2 == 0 else nc.sync
            eng.dma_start(
                out[tt * 128:(tt + 1) * 128, db * 512:(db + 1) * 512],
                o_sb[:],
            )

        # wave A: db 0,1 interleaved chunk-outer (lhsT reused across db — the
        # j=0 chunks run while AllToAll#1 is still in flight)
        opsA = {(db, tt): psum.tile([128, 512], F32, name=f"op{db}_{tt}",
                                    tag="bank")
                for db in (0, 1) for tt in range(4)}
        for j in range(2):
            for c in range(8):
                col = j * 4096 + c * 512
                for tt in range(4):
                    lh = encf_sb[:, col + tt * 128:col + (tt + 1) * 128]
                    for db in (0, 1):
                        nc.tensor.matmul(
                            opsA[(db, tt)][:], lhsT=lh,
                            rhs=oww[db][:, (j * 8 + c) * 512:
                                        (j * 8 + c + 1) * 512],
                            start=(j == 0 and c == 0), stop=(j == 1 and c == 7),
                        )
        for (db, tt), op in opsA.items():
            flush_tile(op, db, tt)

        # wave B: db=2 fully first (oww3 streams in meanwhile), then db=3
        oww[3] = xtp.tile([128, 16 * 512], BF, name="oww3", tag="xt")
        nc.sync.dma_start(oww[3][:], outw2[3])
        for db in (2, 3):
            opsB = {tt: psum.tile([128, 512], F32, name=f"op{db}_{tt}",
                                  tag="bank") for tt in range(4)}
            for tt in range(4):
                oproj_tile(opsB[tt], db, tt)
            for tt in range(4):
                flush_tile(opsB[tt], db, tt)

    nc.compile()
    return nc


_NC_CACHE = None


def _get_nc():
    global _NC_CACHE
    if _NC_CACHE is None:
        _NC_CACHE = _build()
    return _NC_CACHE


def _rope_tables():
    freq_exp = (2.0 / H) * np.arange(HH, dtype=np.float32)
    timescale = (10000.0 ** freq_exp).astype(np.float32)  # [128]
    pos = np.arange(S, dtype=np.float32)
    rad = pos[None, :] / timescale[:, None]  # [128, 2048]
    return np.cos(rad).astype(np.float32), np.sin(rad).astype(np.float32)


def _mask4():
    kk = np.arange(128)[:, None, None]
    rr = np.arange(4)[None, :, None]
    tt = np.arange(512)[None, None, :]
    m = (kk + rr * 128 <= tt)  # [128, 4, 512]
    return np.ascontiguousarray(
        m.reshape(128, 2048).astype(ml_dtypes.bfloat16))


def _prepare_in_maps(x, q_w, kv_w, out_w):
    bf16 = ml_dtypes.bfloat16

    xb = np.asarray(x).reshape(BT, D).astype(bf16)  # [4096 tokens, 2048]
    # [8 tb][128 p][16 dc][512 t]
    xTb_h = np.ascontiguousarray(
        xb.reshape(8, 512, 16, 128).transpose(0, 3, 2, 1).reshape(8, 128, 8192)
    )
    qw_all = np.asarray(q_w).astype(bf16)  # [N, D, H]
    kvw_h = np.ascontiguousarray(
        np.asarray(kv_w)[:, 0].astype(bf16).reshape(2, 16, 128, 256)
        .transpose(2, 0, 1, 3).reshape(128, 8192)
    )
    # out-proj rhs chunks ordered (j, head): col block (j*8+h)*512 of db-slice
    # holds out_w rows [h, j*128:(j+1)*128] x D cols [db*512:(db+1)*512]
    outw_h = np.ascontiguousarray(
        np.asarray(out_w).astype(bf16).reshape(N, 2, 128, 4, 512)
        .transpose(3, 2, 1, 0, 4).reshape(4, 128, 8192)
    )
    cos_t, sin_t = _rope_tables()
    scale = np.float32(1.0 / np.sqrt(H))
    cosq_h = np.ascontiguousarray(cos_t * scale)
    sinq_h = np.ascontiguousarray(sin_t * scale)
    mask_h = _mask4()

    in_maps = []
    for n in range(NCORES):
        g0 = n * TSH
        posk = (np.arange(TSH) + g0) % S
        xkv_h = np.ascontiguousarray(
            xb[g0:g0 + TSH].reshape(512, 16, 128)
            .transpose(2, 1, 0).reshape(128, 8192)
        )
        qw_h = np.ascontiguousarray(
            qw_all[n].reshape(16, 128, 256).transpose(1, 0, 2)
            .reshape(128, 4096)
        )
        in_maps.append({
            "xTb": xTb_h,
            "xkv2": xkv_h,
            "qw2": qw_h,
            "kvw2": kvw_h,
            "outw2": outw_h,
            "cosq": cosq_h,
            "sinq": sinq_h,
            "cosk": np.ascontiguousarray(cos_t[:, posk]),
            "sink": np.ascontiguousarray(sin_t[:, posk]),
            "mask4": mask_h,
        })
    return in_maps


def _assemble_out(results):
    out = np.empty((B, S, D), dtype=np.float32)
    for n in range(NCORES):
        g0 = n * TSH
        out[g0 // S, g0 % S:g0 % S + TSH, :] = results[n]["out"]
    return out


def kernel(x, positions, attn_mask, q_w, kv_w, out_w):
    nc = _get_nc()
    in_maps = _prepare_in_maps(x, q_w, kv_w, out_w)
    res = run_bass_kernel_spmd(nc, in_maps, core_ids=list(range(NCORES)))
    return _assemble_out(res.results)
